# revision 4
# baseline (speedup 1.0000x reference)
"""Trainium2 Bass kernel for multi-head attention with RoPE.

Problem: b=8, n=1024, d_model=768, heads=12, dim_head=64.
Strategy: data parallel over batch — each of the 8 NeuronCores handles one
batch element end-to-end (QKV proj + RoPE + attention + out proj). No
collectives needed.

Per-core math (all in transposed [feature, token] layout so every matmul
contraction sits on the partition axis):
  xT   [768,1024]  = x^T            (bf16, via DMA transpose)
  qkT  [1536,1024] = (Wqk^T x^T)    then RoPE applied in-place per head
  V    [1024, 780] = x Wv           (65 cols/head: 64 v-dims + ones column)
  per head h:
    sT  [1024,1024] = k_h^T.T? . .  S^T tile: S^T[j,i] = sum_d kT[d,j] qT[d,i]
    pT  = exp(sT / 8)               (no max-subtraction; |S/8| <~ 6)
    oT  [65,1024]  += Vaug_h[jtile]^T-style accum over j tiles
                      row 64 = softmax denominators (ones column of Vaug)
    aT_h [64,1024] = oT[0:64] * broadcast(1/oT[64])
  out [1024,768] = aT^T Wout + b    (bias added via DVE broadcast tile)
"""

import os
import numpy as np
import ml_dtypes

N = 1024
D = 768
H = 12
DH = 64
E3 = 2304
KT = 6          # number of 128-row tiles of the model dim (768/128)
NT = 8          # number of 128-token tiles (1024/128)
P = 128
N_CORES = 8
VW = 65         # per-head V width incl. ones column

_CACHE = {}


def _build():
    import concourse.bass as bass
    import concourse.mybir as mybir
    import concourse.tile as tile
    from concourse import bacc

    F32 = mybir.dt.float32
    BF16 = mybir.dt.bfloat16
    Exp = mybir.ActivationFunctionType.Exp

    nc = bacc.Bacc("TRN2", target_bir_lowering=False, debug=False,
                   num_devices=N_CORES)

    x = nc.dram_tensor("x", [N, D], BF16, kind="ExternalInput")
    wqkv = nc.dram_tensor("wqkv", [D, E3], BF16, kind="ExternalInput")
    wout = nc.dram_tensor("wout", [D, D], BF16, kind="ExternalInput")
    cos2 = nc.dram_tensor("cos2", [P, N], F32, kind="ExternalInput")
    sins2 = nc.dram_tensor("sins2", [P, N], F32, kind="ExternalInput")
    biasb = nc.dram_tensor("biasb", [P, D], F32, kind="ExternalInput")
    out = nc.dram_tensor("out", [N, D], F32, kind="ExternalOutput")

    with tile.TileContext(nc) as tc:
        import contextlib
        with contextlib.ExitStack() as ctx:
            persist = ctx.enter_context(tc.tile_pool(name="persist", bufs=1))
            scr = ctx.enter_context(tc.tile_pool(name="scr", bufs=2))
            ptp = ctx.enter_context(tc.tile_pool(name="ptp", bufs=3))
            smallp = ctx.enter_context(tc.tile_pool(name="smallp", bufs=2))
            outp = ctx.enter_context(tc.tile_pool(name="outp", bufs=2))
            dramp = ctx.enter_context(
                tc.tile_pool(name="dram", bufs=2, space="DRAM"))

            # ---- load constants / weights ----
            cos_sb = persist.tile([P, N], F32, tag="cos", name="cos_sb")
            nc.sync.dma_start(cos_sb[:], cos2[:, :])
            sin_sb = persist.tile([P, N], F32, tag="sin", name="sin_sb")
            nc.sync.dma_start(sin_sb[:], sins2[:, :])
            bias_sb = persist.tile([P, D], F32, tag="bias", name="bias_sb")
            nc.sync.dma_start(bias_sb[:], biasb[:, :])

            wq_sb = []
            for k in range(KT):
                t = persist.tile([P, E3], BF16, tag=f"wqkv{k}", name=f"wqkv_sb{k}")
                nc.sync.dma_start(t[:], wqkv[k * P:(k + 1) * P, :])
                wq_sb.append(t)
            wo_sb = []
            for k in range(KT):
                t = persist.tile([P, D], BF16, tag=f"wout{k}", name=f"wout_sb{k}")
                nc.sync.dma_start(t[:], wout[k * P:(k + 1) * P, :])
                wo_sb.append(t)

            # ---- x^T via DMA transpose (bf16) ----
            xT = []
            for t_i in range(KT):
                t = persist.tile([P, N], BF16, tag=f"xT{t_i}", name=f"xT_sb{t_i}")
                nc.sync.dma_start_transpose(t[:], x[:, t_i * P:(t_i + 1) * P])
                xT.append(t)

            # =======================================================
            # Phase B/C: projections (shared PSUM pool, closed after)
            # =======================================================
            qkT = [persist.tile([P, N], BF16, tag=f"qkT{m}", name=f"qkT_sb{m}")
                   for m in range(12)]
            vt = [persist.tile([P, H * VW], BF16, tag=f"vt{n}", name=f"vt_sb{n}")
                  for n in range(NT)]
            aT = [persist.tile([P, N], BF16, tag=f"aT{e}", name=f"aT_sb{e}")
                  for e in range(KT)]

            with tc.tile_pool(name="psBC", bufs=2, space="PSUM") as psBC:
                # q/k projection + RoPE, two heads per m-tile
                for m in range(12):
                    ps = psBC.tile([P, N], F32, tag="ps", name="ps_bc")
                    for ih in range(2):
                        for k in range(KT):
                            nc.tensor.matmul(
                                ps[:, ih * 512:(ih + 1) * 512],
                                lhsT=wq_sb[k][:, m * P:(m + 1) * P],
                                rhs=xT[k][:, ih * 512:(ih + 1) * 512],
                                start=(k == 0), stop=(k == KT - 1))
                    # RoPE: qf = copy(ps); a = qf*cos; b = swap32(qf)*sin_s
                    qf = scr.tile([P, N], F32, tag="qf", name="qf_t")
                    nc.scalar.copy(qf[:], ps[:])
                    qa = scr.tile([P, N], F32, tag="qa", name="qa_t")
                    nc.vector.tensor_mul(qa[:], qf[:], cos_sb[:])
                    qb = scr.tile([P, N], F32, tag="qb", name="qb_t")
                    # sin table is pre-swapped on host so in0/in1 share a
                    # base partition (walrus NCC_IBIR297); only the output
                    # is quadrant-shifted.
                    for blk in range(4):
                        ob = blk * 32
                        ib = (blk ^ 1) * 32  # 0<->32, 64<->96
                        nc.vector.tensor_mul(
                            qb[ob:ob + 32, :], qf[ib:ib + 32, :],
                            sin_sb[ib:ib + 32, :])
                    nc.vector.tensor_add(qkT[m][:], qa[:], qb[:])

                # V projection into per-head 65-wide layout (+ones col)
                for ni in range(NT):
                    ps = psBC.tile([P, N], F32, tag="ps", name="ps_bc")
                    for (c0, cw) in ((0, 512), (512, 256)):
                        for k in range(KT):
                            nc.tensor.matmul(
                                ps[:, c0:c0 + cw],
                                lhsT=xT[k][:, ni * P:(ni + 1) * P],
                                rhs=wq_sb[k][:, 1536 + c0:1536 + c0 + cw],
                                start=(k == 0), stop=(k == KT - 1))
                    # scatter copy into head-strided slots
                    dst8 = vt[ni][:, 0:8 * VW].rearrange(
                        "p (h j) -> p h j", j=VW)[:, :, 0:DH]
                    src8 = ps[:, 0:512].rearrange("p (h j) -> p h j", j=DH)
                    nc.scalar.copy(dst8, src8)
                    dst4 = vt[ni][:, 8 * VW:12 * VW].rearrange(
                        "p (h j) -> p h j", j=VW)[:, :, 0:DH]
                    src4 = ps[:, 512:768].rearrange("p (h j) -> p h j", j=DH)
                    nc.scalar.copy(dst4, src4)
                    ones_cols = vt[ni].rearrange(
                        "p (h j) -> p h j", j=VW)[:, :, DH:VW]
                    nc.gpsimd.memset(ones_cols, 1.0)

            # =======================================================
            # Phase D: attention per head;  Phase E shares psS pool
            # =======================================================
            with (tc.tile_pool(name="psS", bufs=2, space="PSUM") as psS,
                  tc.tile_pool(name="psO", bufs=2, space="PSUM") as psO):
                for h in range(H):
                    qt = qkT[h // 2]
                    kt = qkT[6 + h // 2]
                    rb_ = (h % 2) * DH  # row base within tile
                    o_ps = psO.tile([P, N], F32, tag="ops", name="o_ps_t")
                    for j in range(NT):
                        s_ps = psS.tile([P, N], F32, tag="sps", name="s_ps_t")
                        for ih in range(2):
                            nc.tensor.matmul(
                                s_ps[:, ih * 512:(ih + 1) * 512],
                                lhsT=kt[rb_:rb_ + DH, j * P:(j + 1) * P],
                                rhs=qt[rb_:rb_ + DH, ih * 512:(ih + 1) * 512],
                                start=True, stop=True)
                        pT = ptp.tile([P, N], BF16, tag="pT", name="pT_t")
                        nc.scalar.activation(pT[:], s_ps[:], Exp, scale=0.125)
                        for ih in range(2):
                            nc.tensor.matmul(
                                o_ps[0:VW, ih * 512:(ih + 1) * 512],
                                lhsT=vt[j][:, h * VW:(h + 1) * VW],
                                rhs=pT[:, ih * 512:(ih + 1) * 512],
                                start=(j == 0), stop=(j == NT - 1))
                    # normalization: r = 1/sums; broadcast via DRAM; scale
                    r_sb = smallp.tile([1, N], F32, tag="r", name="r_t")
                    nc.vector.reciprocal(r_sb[:], o_ps[DH:DH + 1, :])
                    r_dr = dramp.tile([1, N], F32, tag="rdr", name="rdr_t")
                    nc.sync.dma_start(r_dr[:], r_sb[:])
                    rb_sb = smallp.tile([DH, N], F32, tag="rb", name="rb_t")
                    nc.sync.dma_start(rb_sb[:], r_dr[:].broadcast_to([DH, N]))
                    nc.vector.tensor_mul(
                        aT[h // 2][rb_:rb_ + DH, :], o_ps[0:DH, :], rb_sb[:])

                # Phase E: output projection + bias
                for it in range(NT):
                    f_ps = psS.tile([P, N], F32, tag="sps", name="s_ps_t")
                    for (c0, cw) in ((0, 512), (512, 256)):
                        for e in range(KT):
                            nc.tensor.matmul(
                                f_ps[:, c0:c0 + cw],
                                lhsT=aT[e][:, it * P:(it + 1) * P],
                                rhs=wo_sb[e][:, c0:c0 + cw],
                                start=(e == 0), stop=(e == KT - 1))
                    o_sb = outp.tile([P, D], F32, tag="osb", name="osb_t")
                    nc.vector.tensor_add(o_sb[:], f_ps[:, 0:D], bias_sb[:])
                    nc.sync.dma_start(out[it * P:(it + 1) * P, :], o_sb[:])

    nc.compile()
    return nc


def _host_tables():
    inv_freq = 1.0 / (10000.0 ** (np.arange(0, DH, 2, dtype=np.float32) / DH))
    t = np.arange(N, dtype=np.float32)
    freqs = np.einsum("i,j->ij", t, inv_freq)          # [N, 32]
    emb = np.concatenate([freqs, freqs], axis=-1)      # [N, 64]
    cosT = np.cos(emb).T.astype(np.float32)            # [64, N]
    sinT = np.sin(emb).T.astype(np.float32)            # [64, N]
    # b-term: out rows 0:32 use -sin (pair d+32), rows 32:64 use +sin
    sins = np.concatenate([-sinT[0:32], sinT[32:64]], axis=0)  # [64, N]
    cos2 = np.concatenate([cosT, cosT], axis=0)        # [128, N]
    sins2 = np.concatenate([sins, sins], axis=0)       # [128, N]
    # pre-swap 32-row blocks (0<->32, 64<->96): the device multiplies
    # qb[ob] = qf[ib] * sin_sb[ib], so sin_sb[ib] must hold sins2[ob].
    sinsw2 = np.concatenate(
        [sins2[32:64], sins2[0:32], sins2[96:128], sins2[64:96]], axis=0)
    return np.ascontiguousarray(cos2), np.ascontiguousarray(sinsw2)


def kernel(x, w_qkv, w_out, b_out):
    from concourse.bass_utils import run_bass_kernel_spmd

    if "nc" not in _CACHE:
        _CACHE["nc"] = _build()
    nc = _CACHE["nc"]

    bf = ml_dtypes.bfloat16
    cos2, sins2 = _host_tables()
    biasb = np.ascontiguousarray(
        np.broadcast_to(np.asarray(b_out, np.float32)[None, :], (P, D)))
    wqkv_b = np.ascontiguousarray(np.asarray(w_qkv, np.float32).astype(bf))
    wout_b = np.ascontiguousarray(np.asarray(w_out, np.float32).astype(bf))

    in_maps = []
    for i in range(N_CORES):
        xi = np.ascontiguousarray(np.asarray(x[i], np.float32).astype(bf))
        in_maps.append({
            "x": xi, "wqkv": wqkv_b, "wout": wout_b,
            "cos2": cos2, "sins2": sins2, "biasb": biasb,
        })

    res = run_bass_kernel_spmd(
        nc, in_maps, list(range(N_CORES)),
        trace=bool(int(os.environ.get("KERNEL_TRACE", "0"))))
    _CACHE["last_result"] = res
    return np.stack([res.results[i]["out"] for i in range(N_CORES)], axis=0)
